# revision 20
# baseline (speedup 1.0000x reference)
"""GATv2 (3-layer, 4-head) on 8 Trainium2 NeuronCores.

Strategy (graph/data parallel, per sharding hint):
- Nodes partitioned across 8 cores by dst (6250 real + 22 pad -> 6272/core).
- Host sorts edges by dst, packs segments (consecutive dsts) into fixed
  supertiles of 49 dsts x <=1024 edge slots. Segment softmax + scatter-add
  become per-supertile matmuls against 0/1 segment matrices; outputs land in
  transposed layout h_T[feat, node] which feeds the next layer's matmuls
  directly. AllGather replicates h between layers.
- Layer 1 needs no gather: host pre-expands x[src] per edge slot (input
  rearrangement only), so Gl comes from a matmul. Layers 2/3 gather xl rows
  via indirect DMA (128 rows/instruction).
- Precision: h chain and xl tables fp32; xr tables stored as f16 hi+lo pairs
  (exact reconstruction inside the PSUM accumulation); segment matrices and
  alpha weights f16 (exact / averaged); attention logits computed in fp32.
"""
import numpy as np

import concourse.bass as bass
import concourse.bacc as bacc
import concourse.mybir as mybir
import concourse.tile as tile
from concourse.bass_utils import run_bass_kernel_spmd

F32 = mybir.dt.float32
F16 = mybir.dt.float16
I32 = mybir.dt.int32
AX = mybir.AxisListType
ALU = mybir.AluOpType
ACTF = mybir.ActivationFunctionType

# ---------------- problem geometry (hardcoded per spec) ----------------
N, E = 50000, 800000
DIM_IN, DIM_H, DIM_OUT, HEADS = 128, 32, 16, 4
HID = DIM_H * HEADS      # 128
FINAL = DIM_OUT * HEADS  # 64
NEG_SLOPE = 0.2

NCORE = 8
NLOC = N // NCORE        # 6250 real nodes per core
SEG = 49                 # dst segments per supertile
NT = 128                 # supertiles per core;  SEG*NT = 6272 = NP
NP = SEG * NT            # padded nodes per core (49*128 = 6272)
NTAB = NCORE * NP        # padded global node count (50176)
ET = 1024                # edge slots per supertile
NB = ET // 128           # gather blocks per supertile (8)
PAD_SRC = 1 << 30        # oob src id for pad slots (gather skipped)
SW = SEG + 1             # segment-matrix width (last col = trash)

FOR_UNROLL = 8
STAGGER = True


# ---------------- host-side preprocessing ----------------
def _preprocess(x, edge_index):
    """Returns per-core host arrays."""
    src = np.asarray(edge_index[0], dtype=np.int64)
    dst = np.asarray(edge_index[1], dtype=np.int64)
    loops = np.arange(N, dtype=np.int64)
    src = np.concatenate([src, loops])
    dst = np.concatenate([dst, loops])

    core = dst // NLOC
    src_p = (src // NLOC) * NP + (src % NLOC)   # padded global ids
    dst_l = dst % NLOC                           # local id on its core

    x = np.asarray(x, dtype=np.float32)

    per_core = []
    for c in range(NCORE):
        m = core == c
        s_c = src_p[m]
        d_c = dst_l[m]
        order = np.argsort(d_c, kind="stable")
        s_c = s_c[order]
        d_c = d_c[order]
        cnt = np.bincount(d_c, minlength=NP)
        # fake edges for pad dsts so no segment is empty
        n_fake = NP - NLOC
        s_c = np.concatenate([s_c, np.full(n_fake, c * NP, dtype=np.int64)])
        d_c = np.concatenate([d_c, np.arange(NLOC, NP, dtype=np.int64)])
        cnt[NLOC:] = 1
        tile_of_dst = np.arange(NP) // SEG
        tile_edges = np.bincount(tile_of_dst, weights=cnt, minlength=NT).astype(np.int64)
        if tile_edges.max() > ET:
            raise RuntimeError(f"supertile overflow: {tile_edges.max()} > {ET}")
        src_slot = np.full((NT, ET), PAD_SRC, dtype=np.int64)
        seg_slot = np.full((NT, ET), SEG, dtype=np.float32)  # pad -> trash seg
        edge_tile = tile_of_dst[d_c]
        t_starts = np.zeros(NT + 1, dtype=np.int64)
        t_starts[1:] = np.cumsum(tile_edges)
        pos_in_tile = np.arange(len(d_c)) - t_starts[edge_tile]
        src_slot[edge_tile, pos_in_tile] = s_c
        seg_slot[edge_tile, pos_in_tile] = (d_c % SEG).astype(np.float32)

        # srcseg int32 [NT*128, 2*NB]: [p, j<NB] = src of slot j*128+p;
        #                              [p, NB+j] = f32 bits of seg id
        srcseg = np.zeros((NT * 128, 2 * NB), dtype=np.int32)
        src_r = src_slot.reshape(NT, NB, 128)          # [t, j, p]
        seg_r = seg_slot.reshape(NT, NB, 128)
        srcseg[:, :NB] = src_r.transpose(0, 2, 1).reshape(NT * 128, NB).astype(np.int32)
        srcseg[:, NB:] = seg_r.transpose(0, 2, 1).reshape(NT * 128, NB).astype(
            np.float32).view(np.int32)
        # xeT f32 [128, NT*ET]: column (t*ET + q) = x[src of slot q] (0 for pads)
        xe = np.zeros((NT * ET, DIM_IN), dtype=np.float32)
        valid = src_slot.reshape(-1) != PAD_SRC
        gsrc = src_slot.reshape(-1)[valid]
        orig = (gsrc // NP) * NLOC + (gsrc % NP)
        xe[valid] = x[orig]
        xeT = np.ascontiguousarray(xe.T)               # [128, NT*ET]
        xT_loc = np.zeros((128, NP), dtype=np.float32)
        xT_loc[:, :NLOC] = x[c * NLOC:(c + 1) * NLOC].T
        per_core.append({"srcseg": srcseg, "xeT": xeT, "xT_loc": xT_loc})
    return per_core


def _consts(Wl1, Wr1, att1, b1, Wl2, Wr2, att2, b2, Wl3, Wr3, att3, b3):
    att1f = np.asarray(att1, np.float32).reshape(-1)
    att2f = np.asarray(att2, np.float32).reshape(-1)
    att3f = np.asarray(att3, np.float32).reshape(-1)
    e4 = np.zeros((HEADS, HID), np.float32)
    for h in range(HEADS):
        e4[h, h * DIM_H:(h + 1) * DIM_H] = 1.0
    e43 = np.zeros((HEADS, FINAL), np.float32)
    for h in range(HEADS):
        e43[h, h * DIM_OUT:(h + 1) * DIM_OUT] = 1.0
    c = {
        "wl1": np.asarray(Wl1, np.float32), "wr1": np.asarray(Wr1, np.float32),
        "wl2": np.asarray(Wl2, np.float32), "wr2": np.asarray(Wr2, np.float32),
        "wl3": np.asarray(Wl3, np.float32), "wr3": np.asarray(Wr3, np.float32),
        "attr1": np.tile(att1f, (128, 1)),
        "attr2": np.tile(att2f, (128, 1)),
        "attr3": np.tile(att3f, (128, 1)),
        "b1c": np.asarray(b1, np.float32).reshape(HID, 1),
        "b2c": np.asarray(b2, np.float32).reshape(HID, 1),
        "b3r": np.tile(np.asarray(b3, np.float32).reshape(1, FINAL), (128, 1)),
        "iota": np.tile(np.arange(128, dtype=np.float32)[None, :], (128, 1)),
        "ident": np.eye(128, dtype=np.float16),
        "ident32": np.eye(128, dtype=np.float32),
        "e4": e4, "e43": e43,
    }
    return c


# ---------------- device kernel ----------------
def _build():
    nc = bacc.Bacc("TRN2", target_bir_lowering=False, debug=False,
                   enable_asserts=False, num_devices=NCORE)

    def din(name, shape, dt):
        return nc.dram_tensor(name, shape, dt, kind="ExternalInput").ap()

    srcseg = din("srcseg", [NT * 128, 2 * NB], I32)
    xeT = din("xeT", [128, NT * ET], F32)
    xT_loc = din("xT_loc", [128, NP], F32)
    wl1 = din("wl1", [128, HID], F32); wr1 = din("wr1", [128, HID], F32)
    wl2 = din("wl2", [128, HID], F32); wr2 = din("wr2", [128, HID], F32)
    wl3 = din("wl3", [128, FINAL], F32); wr3 = din("wr3", [128, FINAL], F32)
    attr1 = din("attr1", [128, HID], F32)
    attr2 = din("attr2", [128, HID], F32)
    attr3 = din("attr3", [128, FINAL], F32)
    b1c = din("b1c", [HID, 1], F32)
    b2c = din("b2c", [HID, 1], F32)
    b3r = din("b3r", [128, FINAL], F32)
    iota = din("iota", [128, 128], F32)
    ident = din("ident", [128, 128], F16)
    ident32 = din("ident32", [128, 128], F32)
    e4 = din("e4", [HEADS, HID], F32)
    e43 = din("e43", [HEADS, FINAL], F32)

    outp = nc.dram_tensor("outp", [NP, FINAL], F32, kind="ExternalOutput").ap()

    with tile.TileContext(nc) as tc:
        with tc.tile_pool(name="cst", bufs=1) as cst, \
             tc.tile_pool(name="hbuf", bufs=1) as hbuf, \
             tc.tile_pool(name="sb", bufs=3) as sb, \
             tc.tile_pool(name="p2", bufs=2, space="PSUM") as p2, \
             tc.tile_pool(name="p1", bufs=1, space="PSUM") as p1, \
             tc.tile_pool(name="dram", bufs=1, space="DRAM") as dram:

            def load_const(apx, dt, tag=None):
                t = cst.tile(list(apx.shape), dt, tag=tag or apx.tensor.name)
                nc.sync.dma_start(out=t[:], in_=apx[:])
                return t

            wl1_s = load_const(wl1, F32); wr1_s = load_const(wr1, F32)
            wl2_s = load_const(wl2, F32); wr2_s = load_const(wr2, F32)
            wl3_s = load_const(wl3, F32); wr3_s = load_const(wr3, F32)
            attr1_s = load_const(attr1, F32); attr2_s = load_const(attr2, F32)
            attr3_s = load_const(attr3, F32)
            b1c_s = load_const(b1c, F32); b2c_s = load_const(b2c, F32)
            b3r_s = load_const(b3r, F32)
            iota_s = load_const(iota, F32)
            ident_s = load_const(ident, F16)
            ident32_s = load_const(ident32, F32)
            e4_s = load_const(e4, F32); e43_s = load_const(e43, F32)
            zeros_s = cst.tile([128, SW], F32, tag="zeros")
            nc.vector.memset(zeros_s[:], 0.0)

            hT = hbuf.tile([128, NP], F32, tag="hT")

            xl2 = dram.tile([NTAB, HID], F32, tag="xl2")
            xl3 = dram.tile([NTAB, FINAL], F32, tag="xl3")
            xr_hi = [dram.tile([NP + 64, HID], F16, name=f"xrh{l}", tag=f"xrh{l}") for l in range(3)]
            xr_lo = [dram.tile([NP + 64, HID], F16, name=f"xrl{l}", tag=f"xrl{l}") for l in range(3)]
            cc_in1 = dram.tile([128 * NP], F32, tag="cc_in1")
            cc_out1 = dram.tile([NCORE * 128, NP], F32, tag="cc_out1")
            cc_in2 = dram.tile([128 * NP], F32, tag="cc_in2")
            cc_out2 = dram.tile([NCORE * 128, NP], F32, tag="cc_out2")
            opreT = dram.tile([FINAL, NP], F32, tag="opreT")

            # zero the xr over-read tails
            ztail = sb.tile([64, HID], F16, tag="ztail")
            nc.vector.memset(ztail[:], 0.0)
            for l in range(3):
                nc.sync.dma_start(out=xr_hi[l][NP:NP + 64, :], in_=ztail[:])
                nc.sync.dma_start(out=xr_lo[l][NP:NP + 64, :], in_=ztail[:])

            # ================= edge phase =================
            def edge_tile(i, layer):
                F = HID if layer < 3 else FINAL
                attr_s = (attr1_s, attr2_s, attr3_s)[layer - 1]
                ss = sb.tile([128, 2 * NB], I32, tag="ss")
                nc.sync.dma_start(out=ss[:], in_=srcseg[bass.ds(i * 128, 128), :])
                segf = ss[:].bitcast(F32)[:, NB:2 * NB]        # [128, NB] f32

                # --- Gl [128e, NB*F] f32 ---
                GRP = 512 // F
                ngrp = (NB + GRP - 1) // GRP
                gl = sb.tile([128, NB * F], F32, tag="gl")
                if layer == 1:
                    xt = sb.tile([128, ET], F32, tag="xt")
                    nc.sync.dma_start(out=xt[:], in_=xeT[:, bass.ds(i * ET, ET)])
                    for g in range(ngrp):
                        nblk = min(GRP, NB - g * GRP)
                        gp = p2.tile([128, 512], F32, tag="pbig", space="PSUM")
                        for jj in range(nblk):
                            j = g * GRP + jj
                            nc.tensor.matmul(
                                gp[:, jj * F:(jj + 1) * F],
                                lhsT=xt[:, j * 128:(j + 1) * 128],
                                rhs=wl1_s[:], start=True, stop=True)
                        nc.vector.tensor_copy(gl[:, g * GRP * F:(g * GRP + nblk) * F],
                                              gp[:, :nblk * F])
                else:
                    tbl = xl2 if layer == 2 else xl3
                    for j in range(NB):
                        nc.gpsimd.indirect_dma_start(
                            out=gl[:, j * F:(j + 1) * F], out_offset=None,
                            in_=tbl[:],
                            in_offset=bass.IndirectOffsetOnAxis(ap=ss[:, j:j + 1], axis=0),
                            bounds_check=NTAB - 1, oob_is_err=False)

                # --- segment matrix S [128e, NB, SW] f16 ---
                S = sb.tile([128, NB, SW], F16, tag="S")
                nc.vector.tensor_tensor(
                    out=S[:],
                    in0=segf.rearrange("p (b one) -> p b one", one=1).to_broadcast([128, NB, SW]),
                    in1=iota_s[:, :SW].rearrange("p (one s) -> p one s", one=1)
                        .to_broadcast([128, NB, SW]),
                    op=ALU.is_equal)

                # --- S_T via PE transpose: st [SW, NB*128] f16 ---
                st = sb.tile([SW, NB * 128], F16, tag="st")
                nsgrp = (NB + 3) // 4
                for g in range(nsgrp):
                    nblk = min(4, NB - g * 4)
                    sp = p2.tile([SW, 512], F16, tag="psp", space="PSUM")
                    for jj in range(nblk):
                        j = g * 4 + jj
                        nc.tensor.transpose(sp[:, jj * 128:(jj + 1) * 128],
                                            S[:, j, :], ident_s[:])
                    nc.vector.tensor_copy(st[:, g * 512:g * 512 + nblk * 128],
                                          sp[:, :nblk * 128])

                # --- xr_u rows (hi/lo) for this tile's dsts ---
                hi_t, lo_t = xr_hi[layer - 1], xr_lo[layer - 1]
                xu = sb.tile([SW, 2 * HID], F16, tag="xu")
                nc.sync.dma_start(out=xu[:, :F], in_=hi_t[bass.ds(i * SEG, SW), :F])
                nc.sync.dma_start(out=xu[:, HID:HID + F], in_=lo_t[bass.ds(i * SEG, SW), :F])

                # --- m = Gl + S_T.T @ (xr_hi + xr_lo); leaky-relu (DVE) ---
                lr = sb.tile([128, NB * F], F32, tag="lr")
                for g in range(ngrp):
                    nblk = min(GRP, NB - g * GRP)
                    mp = p2.tile([128, 512], F32, tag="pbig", space="PSUM")
                    for jj in range(nblk):
                        j = g * GRP + jj
                        nc.tensor.matmul(mp[:, jj * F:(jj + 1) * F],
                                         lhsT=st[:, j * 128:(j + 1) * 128],
                                         rhs=xu[:, :F], start=True, stop=False)
                        nc.tensor.matmul(mp[:, jj * F:(jj + 1) * F],
                                         lhsT=st[:, j * 128:(j + 1) * 128],
                                         rhs=xu[:, HID:HID + F], start=False, stop=True)
                    gsl = slice(g * GRP * F, (g * GRP + nblk) * F)
                    msb = sb.tile([128, 512], F32, tag="msb")
                    nc.vector.tensor_add(msb[:, :nblk * F], gl[:, gsl],
                                         mp[:, :nblk * F])
                    nc.vector.scalar_tensor_tensor(
                        out=lr[:, gsl], in0=msb[:, :nblk * F], scalar=NEG_SLOPE,
                        in1=msb[:, :nblk * F], op0=ALU.mult, op1=ALU.max)

                # --- e = per-head dot with att (DVE), exp (ACT) ---
                C = F // HEADS
                tm = sb.tile([128, NB * F], F32, tag="tm")
                nc.vector.tensor_tensor(
                    out=tm[:].rearrange("p (b h c) -> p b h c", b=NB, h=HEADS),
                    in0=lr[:].rearrange("p (b h c) -> p b h c", b=NB, h=HEADS),
                    in1=attr_s[:].rearrange("p (one h c) -> p one h c", one=1, h=HEADS)
                        .to_broadcast([128, NB, HEADS, C]),
                    op=ALU.mult)
                ev = sb.tile([128, NB * HEADS], F32, tag="ev")
                nc.vector.reduce_sum(
                    ev[:].rearrange("p (b h one) -> p b h one", b=NB, one=1),
                    tm[:].rearrange("p (b h c) -> p b h c", b=NB, h=HEADS),
                    axis=AX.X)
                exs = sb.tile([128, NB * HEADS], F16, tag="exs")
                nc.scalar.activation(exs[:], ev[:], ACTF.Exp)

                # --- P = Gl * ex (head-broadcast), f16 ---
                pp = sb.tile([128, NB * F], F16, tag="pp")
                nc.vector.tensor_tensor(
                    out=pp[:].rearrange("p (b h c) -> p b h c", b=NB, h=HEADS),
                    in0=gl[:].rearrange("p (b h c) -> p b h c", b=NB, h=HEADS),
                    in1=exs[:].rearrange("p (b h one) -> p b h one", b=NB, one=1)
                        .to_broadcast([128, NB, HEADS, C]),
                    op=ALU.mult)

                # --- segment sums ---
                otp = p2.tile([F, SW], F32, tag="otp", space="PSUM")
                dnp = p1.tile([HEADS, SW], F32, tag="dnp", space="PSUM")
                for j in range(NB):
                    nc.tensor.matmul(otp[:], lhsT=pp[:, j * F:(j + 1) * F],
                                     rhs=S[:, j, :],
                                     start=(j == 0), stop=(j == NB - 1))
                for j in range(NB):
                    nc.tensor.matmul(dnp[:], lhsT=exs[:, j * HEADS:(j + 1) * HEADS],
                                     rhs=S[:, j, :],
                                     start=(j == 0), stop=(j == NB - 1))
                rd = sb.tile([HEADS, SW], F32, tag="rd")
                nc.vector.reciprocal(rd[:], dnp[:])
                dxp = p1.tile([F, SW], F32, tag="dxp", space="PSUM")
                e4c = e4_s if layer < 3 else e43_s
                nc.tensor.matmul(dxp[:], lhsT=e4c[:], rhs=rd[:], start=True, stop=True)
                dxs = sb.tile([F, SW], F32, tag="dxs")
                nc.vector.tensor_copy(dxs[:], dxp[:])
                z = sb.tile([F, SW], F32, tag="z")
                nc.vector.tensor_tensor(out=z[:], in0=otp[:], in1=dxs[:], op=ALU.mult)

                if layer < 3:
                    bc = b1c_s if layer == 1 else b2c_s
                    t1 = sb.tile([F, SEG], F32, tag="t1")
                    nc.vector.scalar_tensor_tensor(
                        out=t1[:], in0=z[:, :SEG], scalar=bc[:, :1],
                        in1=zeros_s[:F, :SEG], op0=ALU.add, op1=ALU.min)
                    t2 = sb.tile([F, SEG], F32, tag="t2")
                    nc.scalar.activation(t2[:], t1[:], ACTF.Exp)
                    t3 = sb.tile([F, SEG], F32, tag="t3")
                    nc.vector.scalar_tensor_tensor(
                        out=t3[:], in0=z[:, :SEG], scalar=bc[:, :1],
                        in1=zeros_s[:F, :SEG], op0=ALU.add, op1=ALU.max)
                    nc.vector.scalar_tensor_tensor(
                        out=hT[:, bass.ds(i * SEG, SEG)], in0=t2[:], scalar=-1.0,
                        in1=t3[:], op0=ALU.add, op1=ALU.add)
                else:
                    nc.sync.dma_start(out=opreT[:, bass.ds(i * SEG, SEG)],
                                      in_=z[:, :SEG])

            # ================= table phases =================
            def xr_phase(wr_s, li, F):
                hi_t, lo_t = xr_hi[li], xr_lo[li]
                for c in range(NP // 128):
                    p = p2.tile([128, F], F32, tag="pbig", space="PSUM")
                    nc.tensor.matmul(p[:], lhsT=hT[:, c * 128:(c + 1) * 128],
                                     rhs=wr_s[:], start=True, stop=True)
                    shi = sb.tile([128, F], F16, tag="shi")
                    nc.vector.tensor_copy(shi[:], p[:])
                    slo = sb.tile([128, F], F16, tag="slo")
                    nc.vector.tensor_tensor(out=slo[:], in0=p[:], in1=shi[:],
                                            op=ALU.subtract)
                    nc.sync.dma_start(out=hi_t[c * 128:(c + 1) * 128, :F], in_=shi[:])
                    nc.sync.dma_start(out=lo_t[c * 128:(c + 1) * 128, :F], in_=slo[:])

            def xl_phase(cc_out, wl_s, xlt, F):
                def body(ci):
                    for r in range(NCORE):
                        htc = sb.tile([128, 128], F32, tag="htc")
                        nc.sync.dma_start(
                            out=htc[:],
                            in_=cc_out[r * 128:(r + 1) * 128, bass.ds(ci * 128, 128)])
                        p = p2.tile([128, F], F32, tag="pbig", space="PSUM")
                        nc.tensor.matmul(p[:], lhsT=htc[:], rhs=wl_s[:],
                                         start=True, stop=True)
                        s = sb.tile([128, F], F32, tag="xls")
                        nc.vector.tensor_copy(s[:], p[:])
                        nc.sync.dma_start(
                            out=xlt[bass.ds(r * NP + ci * 128, 128), :], in_=s[:])
                nchunks = NP // 128
                bulk = (nchunks // 8) * 8
                if bulk >= 8:
                    with tc.For_i(0, bulk, 8, staggered_reset=STAGGER) as ci:
                        for u in range(8):
                            body(ci + u)
                else:
                    bulk = 0
                for cc in range(bulk, nchunks):
                    body(cc)

            def edge_loop(layer):
                with tc.For_i(0, NT, FOR_UNROLL, staggered_reset=STAGGER) as i0:
                    for u in range(FOR_UNROLL):
                        edge_tile(i0 + u, layer)

            # ================= layers =================
            nc.sync.dma_start(out=hT[:], in_=xT_loc[:])
            xr_phase(wr1_s, 0, HID)

            edge_loop(1)

            nc.sync.dma_start(out=cc_in1[:].rearrange("(p n) -> p n", p=128), in_=hT[:])
            nc.gpsimd.collective_compute(
                "AllGather", ALU.bypass,
                ins=[cc_in1.opt()], outs=[cc_out1.opt()],
                replica_groups=[list(range(NCORE))])
            xr_phase(wr2_s, 1, HID)
            xl_phase(cc_out1, wl2_s, xl2, HID)

            edge_loop(2)

            nc.sync.dma_start(out=cc_in2[:].rearrange("(p n) -> p n", p=128), in_=hT[:])
            nc.gpsimd.collective_compute(
                "AllGather", ALU.bypass,
                ins=[cc_in2.opt()], outs=[cc_out2.opt()],
                replica_groups=[list(range(NCORE))])
            xr_phase(wr3_s, 2, FINAL)
            xl_phase(cc_out2, wl3_s, xl3, FINAL)

            edge_loop(3)

            # ================= final: bias + log_softmax =================
            for c in range(NP // 128):
                ot = sb.tile([FINAL, 128], F32, tag="ot")
                nc.sync.dma_start(out=ot[:], in_=opreT[:, c * 128:(c + 1) * 128])
                tp = p2.tile([128, FINAL], F32, tag="pbig", space="PSUM")
                nc.tensor.transpose(tp[:], ot[:], ident32_s[:FINAL, :FINAL])
                t = sb.tile([128, FINAL], F32, tag="t")
                nc.vector.tensor_add(t[:], tp[:], b3r_s[:])
                nmx = sb.tile([128, 1], F32, tag="nmx")
                nc.vector.reduce_max(nmx[:], t[:], axis=AX.X, negate=True)
                em = sb.tile([128, FINAL], F32, tag="em")
                sm = sb.tile([128, 1], F32, tag="sm")
                nc.scalar.activation(em[:], t[:], ACTF.Exp, bias=nmx[:, :1],
                                     accum_out=sm[:])
                ln = sb.tile([128, 1], F32, tag="ln")
                nc.scalar.activation(ln[:], sm[:], ACTF.Ln)
                o = sb.tile([128, FINAL], F32, tag="o")
                nc.vector.scalar_tensor_tensor(
                    out=o[:], in0=t[:], scalar=nmx[:, :1],
                    in1=ln[:].to_broadcast([128, FINAL]),
                    op0=ALU.add, op1=ALU.subtract)
                nc.sync.dma_start(out=outp[c * 128:(c + 1) * 128, :], in_=o[:])

    nc.compile()
    return nc


_CACHED = {}
TRACE = False
LAST_EXEC_NS = None


def kernel(x, edge_index, Wl1, Wr1, att1, b1, Wl2, Wr2, att2, b2,
           Wl3, Wr3, att3, b3):
    per_core = _preprocess(x, edge_index)
    consts = _consts(Wl1, Wr1, att1, b1, Wl2, Wr2, att2, b2, Wl3, Wr3, att3, b3)

    if "nc" not in _CACHED:
        _CACHED["nc"] = _build()
    nc = _CACHED["nc"]

    in_maps = []
    for c in range(NCORE):
        m = dict(consts)
        m.update(per_core[c])
        in_maps.append(m)

    global LAST_EXEC_NS
    res = run_bass_kernel_spmd(nc, in_maps, core_ids=list(range(NCORE)),
                               trace=TRACE)
    LAST_EXEC_NS = res.exec_time_ns
    out = np.zeros((N, FINAL), dtype=np.float32)
    for c in range(NCORE):
        out[c * NLOC:(c + 1) * NLOC] = res.results[c]["outp"][:NLOC]
    return out


# revision 23
# speedup vs baseline: 1.1904x; 1.1904x over previous
"""GATv2 (3-layer, 4-head) on 8 Trainium2 NeuronCores.

Strategy (graph/data parallel, per sharding hint):
- Nodes partitioned across 8 cores by dst (6250 real + 22 pad -> 6272/core).
- Host sorts edges by dst, packs segments (consecutive dsts) into fixed
  supertiles of 49 dsts x <=1024 edge slots. Segment softmax + scatter-add
  become per-supertile matmuls against 0/1 segment matrices; outputs land in
  transposed layout h_T[feat, node] which feeds the next layer's matmuls
  directly. AllGather replicates h between layers.
- Layer 1 needs no gather: host pre-expands x[src] per edge slot (input
  rearrangement only), so Gl comes from a matmul. Layers 2/3 gather xl rows
  via indirect DMA (128 rows/instruction).
- Precision: h chain and xl tables fp32; xr tables stored as f16 hi+lo pairs
  (exact reconstruction inside the PSUM accumulation); segment matrices and
  alpha weights f16 (exact / averaged); attention logits computed in fp32.
"""
import numpy as np

import concourse.bass as bass
import concourse.bacc as bacc
import concourse.mybir as mybir
import concourse.tile as tile
from concourse.bass_utils import run_bass_kernel_spmd

F32 = mybir.dt.float32
F16 = mybir.dt.float16
I32 = mybir.dt.int32
AX = mybir.AxisListType
ALU = mybir.AluOpType
ACTF = mybir.ActivationFunctionType

# ---------------- problem geometry (hardcoded per spec) ----------------
N, E = 50000, 800000
DIM_IN, DIM_H, DIM_OUT, HEADS = 128, 32, 16, 4
HID = DIM_H * HEADS      # 128
FINAL = DIM_OUT * HEADS  # 64
NEG_SLOPE = 0.2

NCORE = 8
NLOC = N // NCORE        # 6250 real nodes per core
SEG = 49                 # dst segments per supertile
NT = 128                 # supertiles per core;  SEG*NT = 6272 = NP
NP = SEG * NT            # padded nodes per core (49*128 = 6272)
NTAB = NCORE * NP        # padded global node count (50176)
ET = 1024                # edge slots per supertile
NB = ET // 128           # gather blocks per supertile (8)
PAD_SRC = 1 << 30        # oob src id for pad slots (gather skipped)
SW = SEG + 1             # segment-matrix width (last col = trash)

FOR_UNROLL = 8
STAGGER = True
USE_DMA_GATHER = True
BETA = 25088             # table split row (balanced halves, both int16-addressable)
CA = 4                   # gather blocks for half A (derived/verified in _preprocess)
CB = 4                   # gather blocks for half B
I16 = mybir.dt.int16


# ---------------- host-side preprocessing ----------------
def _preprocess(x, edge_index):
    """Returns per-core host arrays."""
    src = np.asarray(edge_index[0], dtype=np.int64)
    dst = np.asarray(edge_index[1], dtype=np.int64)
    loops = np.arange(N, dtype=np.int64)
    src = np.concatenate([src, loops])
    dst = np.concatenate([dst, loops])

    core = dst // NLOC
    src_p = (src // NLOC) * NP + (src % NLOC)   # padded global ids
    dst_l = dst % NLOC                           # local id on its core

    x = np.asarray(x, dtype=np.float32)

    per_core = []
    for c in range(NCORE):
        m = core == c
        s_c = src_p[m]
        d_c = dst_l[m]
        order = np.argsort(d_c, kind="stable")
        s_c = s_c[order]
        d_c = d_c[order]
        cnt = np.bincount(d_c, minlength=NP)
        # fake edges for pad dsts so no segment is empty
        n_fake = NP - NLOC
        s_c = np.concatenate([s_c, np.full(n_fake, c * NP, dtype=np.int64)])
        d_c = np.concatenate([d_c, np.arange(NLOC, NP, dtype=np.int64)])
        cnt[NLOC:] = 1
        tile_of_dst = np.arange(NP) // SEG
        tile_edges = np.bincount(tile_of_dst, weights=cnt, minlength=NT).astype(np.int64)
        if tile_edges.max() > ET:
            raise RuntimeError(f"supertile overflow: {tile_edges.max()} > {ET}")
        src_slot = np.full((NT, ET), PAD_SRC, dtype=np.int64)
        seg_slot = np.full((NT, ET), SEG, dtype=np.float32)  # pad -> trash seg
        edge_tile = tile_of_dst[d_c]
        t_starts = np.zeros(NT + 1, dtype=np.int64)
        t_starts[1:] = np.cumsum(tile_edges)
        pos_in_tile = np.arange(len(d_c)) - t_starts[edge_tile]
        src_slot[edge_tile, pos_in_tile] = s_c
        seg_slot[edge_tile, pos_in_tile] = (d_c % SEG).astype(np.float32)

        if USE_DMA_GATHER:
            # reorder each tile's slots: half-A edges, then half-B, pads last
            cls = np.where(src_slot == PAD_SRC, 2, (src_slot >= BETA).astype(np.int64))
            order = np.argsort(cls, axis=1, kind="stable")
            src_slot = np.take_along_axis(src_slot, order, axis=1)
            seg_slot = np.take_along_axis(seg_slot, order, axis=1)
            nA = (cls == 0).sum(1)
            nB = (cls == 1).sum(1)
            if nA.max() > CA * 128 or nB.max() > CB * 128:
                raise RuntimeError(f"half overflow: {nA.max()} {nB.max()}")
            # repack: A at slots [0, CA*128), B at [CA*128, ...)
            new_src = np.full((NT, ET), PAD_SRC, dtype=np.int64)
            new_seg = np.full((NT, ET), SEG, dtype=np.float32)
            for t in range(NT):
                a, b = nA[t], nB[t]
                new_src[t, :a] = src_slot[t, :a]
                new_seg[t, :a] = seg_slot[t, :a]
                new_src[t, CA * 128:CA * 128 + b] = src_slot[t, a:a + b]
                new_seg[t, CA * 128:CA * 128 + b] = seg_slot[t, a:a + b]
            src_slot, seg_slot = new_src, new_seg
            # int16 wrapped gather indices [128, (CA+CB)*8]
            gidx = np.zeros((NT * 128, (CA + CB) * 8), dtype=np.int16)
            for t in range(NT):
                ia = np.full(CA * 128, -1, np.int64)
                ia[:nA[t]] = src_slot[t, :nA[t]]
                ib = np.full(CB * 128, -1, np.int64)
                ib[:nB[t]] = src_slot[t, CA * 128:CA * 128 + nB[t]] - BETA
                ib[nB[t]:] = -1
                wa = np.tile(ia.reshape(CA * 8, 16).T, (8, 1))   # [128, CA*8]
                wb = np.tile(ib.reshape(CB * 8, 16).T, (8, 1))
                gidx[t * 128:(t + 1) * 128, :CA * 8] = wa.astype(np.int16)
                gidx[t * 128:(t + 1) * 128, CA * 8:] = wb.astype(np.int16)
        # srcseg int32 [NT*128, 2*NB]: [p, j<NB] = src of slot j*128+p;
        #                              [p, NB+j] = f32 bits of seg id
        srcseg = np.zeros((NT * 128, 2 * NB), dtype=np.int32)
        src_r = src_slot.reshape(NT, NB, 128)          # [t, j, p]
        seg_r = seg_slot.reshape(NT, NB, 128)
        srcseg[:, :NB] = src_r.transpose(0, 2, 1).reshape(NT * 128, NB).astype(np.int32)
        srcseg[:, NB:] = seg_r.transpose(0, 2, 1).reshape(NT * 128, NB).astype(
            np.float32).view(np.int32)
        # xeT f32 [128, NT*ET]: column (t*ET + q) = x[src of slot q] (0 for pads)
        xe = np.zeros((NT * ET, DIM_IN), dtype=np.float32)
        valid = src_slot.reshape(-1) != PAD_SRC
        gsrc = src_slot.reshape(-1)[valid]
        orig = (gsrc // NP) * NLOC + (gsrc % NP)
        xe[valid] = x[orig]
        xeT = np.ascontiguousarray(xe.T)               # [128, NT*ET]
        xT_loc = np.zeros((128, NP), dtype=np.float32)
        xT_loc[:, :NLOC] = x[c * NLOC:(c + 1) * NLOC].T
        d = {"srcseg": srcseg, "xeT": xeT, "xT_loc": xT_loc}
        if USE_DMA_GATHER:
            d["gidx"] = gidx
        per_core.append(d)
    return per_core


def _consts(Wl1, Wr1, att1, b1, Wl2, Wr2, att2, b2, Wl3, Wr3, att3, b3):
    att1f = np.asarray(att1, np.float32).reshape(-1)
    att2f = np.asarray(att2, np.float32).reshape(-1)
    att3f = np.asarray(att3, np.float32).reshape(-1)
    e4 = np.zeros((HEADS, HID), np.float32)
    for h in range(HEADS):
        e4[h, h * DIM_H:(h + 1) * DIM_H] = 1.0
    e43 = np.zeros((HEADS, FINAL), np.float32)
    for h in range(HEADS):
        e43[h, h * DIM_OUT:(h + 1) * DIM_OUT] = 1.0
    c = {
        "wl1": np.asarray(Wl1, np.float32), "wr1": np.asarray(Wr1, np.float32),
        "wl2": np.asarray(Wl2, np.float32), "wr2": np.asarray(Wr2, np.float32),
        "wl3": np.asarray(Wl3, np.float32), "wr3": np.asarray(Wr3, np.float32),
        "attr1": np.tile(att1f, (128, 1)),
        "attr2": np.tile(att2f, (128, 1)),
        "attr3": np.tile(att3f, (128, 1)),
        "b1c": np.asarray(b1, np.float32).reshape(HID, 1),
        "b2c": np.asarray(b2, np.float32).reshape(HID, 1),
        "b3r": np.tile(np.asarray(b3, np.float32).reshape(1, FINAL), (128, 1)),
        "iota": np.tile(np.arange(128, dtype=np.float32)[None, :], (128, 1)),
        "ident": np.eye(128, dtype=np.float16),
        "ident32": np.eye(128, dtype=np.float32),
        "e4": e4, "e43": e43,
    }
    return c


# ---------------- device kernel ----------------
def _build():
    nc = bacc.Bacc("TRN2", target_bir_lowering=False, debug=False,
                   enable_asserts=False, num_devices=NCORE)

    def din(name, shape, dt):
        return nc.dram_tensor(name, shape, dt, kind="ExternalInput").ap()

    srcseg = din("srcseg", [NT * 128, 2 * NB], I32)
    if USE_DMA_GATHER:
        gidx = din("gidx", [NT * 128, (CA + CB) * 8], I16)
    xeT = din("xeT", [128, NT * ET], F32)
    xT_loc = din("xT_loc", [128, NP], F32)
    wl1 = din("wl1", [128, HID], F32); wr1 = din("wr1", [128, HID], F32)
    wl2 = din("wl2", [128, HID], F32); wr2 = din("wr2", [128, HID], F32)
    wl3 = din("wl3", [128, FINAL], F32); wr3 = din("wr3", [128, FINAL], F32)
    attr1 = din("attr1", [128, HID], F32)
    attr2 = din("attr2", [128, HID], F32)
    attr3 = din("attr3", [128, FINAL], F32)
    b1c = din("b1c", [HID, 1], F32)
    b2c = din("b2c", [HID, 1], F32)
    b3r = din("b3r", [128, FINAL], F32)
    iota = din("iota", [128, 128], F32)
    ident = din("ident", [128, 128], F16)
    ident32 = din("ident32", [128, 128], F32)
    e4 = din("e4", [HEADS, HID], F32)
    e43 = din("e43", [HEADS, FINAL], F32)

    outp = nc.dram_tensor("outp", [NP, FINAL], F32, kind="ExternalOutput").ap()

    with tile.TileContext(nc) as tc:
        with tc.tile_pool(name="cst", bufs=1) as cst, \
             tc.tile_pool(name="hbuf", bufs=1) as hbuf, \
             tc.tile_pool(name="sb", bufs=3) as sb, \
             tc.tile_pool(name="p2", bufs=2, space="PSUM") as p2, \
             tc.tile_pool(name="p1", bufs=1, space="PSUM") as p1, \
             tc.tile_pool(name="dram", bufs=1, space="DRAM") as dram:

            def load_const(apx, dt, tag=None):
                t = cst.tile(list(apx.shape), dt, tag=tag or apx.tensor.name)
                nc.sync.dma_start(out=t[:], in_=apx[:])
                return t

            wl1_s = load_const(wl1, F32); wr1_s = load_const(wr1, F32)
            wl2_s = load_const(wl2, F32); wr2_s = load_const(wr2, F32)
            wl3_s = load_const(wl3, F32); wr3_s = load_const(wr3, F32)
            attr1_s = load_const(attr1, F32); attr2_s = load_const(attr2, F32)
            attr3_s = load_const(attr3, F32)
            b1c_s = load_const(b1c, F32); b2c_s = load_const(b2c, F32)
            b3r_s = load_const(b3r, F32)
            iota_s = load_const(iota, F32)
            ident_s = load_const(ident, F16)
            ident32_s = load_const(ident32, F32)
            e4_s = load_const(e4, F32); e43_s = load_const(e43, F32)
            zeros_s = cst.tile([128, SW], F32, tag="zeros")
            nc.vector.memset(zeros_s[:], 0.0)

            hT = hbuf.tile([128, NP], F32, tag="hT")
            if USE_DMA_GATHER:
                from concourse.library_config import mlp as _mlp
                nc.gpsimd.load_library(_mlp)
                for _ in range(3):   # pre-zero gl pool slots (pad slots stay finite)
                    gz = sb.tile([128, NB * HID], F32, tag="gl")
                    nc.vector.memset(gz[:], 0.0)

            xl2 = dram.tile([NTAB, HID], F32, tag="xl2")
            xl3 = dram.tile([NTAB, FINAL], F32, tag="xl3")
            xr_hi = [dram.tile([NP + 64, HID], F16, name=f"xrh{l}", tag=f"xrh{l}") for l in range(3)]
            xr_lo = [dram.tile([NP + 64, HID], F16, name=f"xrl{l}", tag=f"xrl{l}") for l in range(3)]
            cc_in1 = dram.tile([128 * NP], F32, tag="cc_in1")
            cc_out1 = dram.tile([NCORE * 128, NP], F32, tag="cc_out1", addr_space="Shared")
            cc_in2 = dram.tile([128 * NP], F32, tag="cc_in2")
            cc_out2 = dram.tile([NCORE * 128, NP], F32, tag="cc_out2", addr_space="Shared")
            opreT = dram.tile([FINAL, NP], F32, tag="opreT")

            # zero the xr over-read tails
            ztail = sb.tile([64, HID], F16, tag="ztail")
            nc.vector.memset(ztail[:], 0.0)
            for l in range(3):
                nc.sync.dma_start(out=xr_hi[l][NP:NP + 64, :], in_=ztail[:])
                nc.sync.dma_start(out=xr_lo[l][NP:NP + 64, :], in_=ztail[:])

            # ================= edge phase =================
            def edge_tile(i, layer):
                F = HID if layer < 3 else FINAL
                attr_s = (attr1_s, attr2_s, attr3_s)[layer - 1]
                ew = nc.gpsimd if layer == 1 else nc.vector  # POOL idle in L1
                ss = sb.tile([128, 2 * NB], I32, tag="ss")
                nc.sync.dma_start(out=ss[:], in_=srcseg[bass.ds(i * 128, 128), :])
                segf = ss[:].bitcast(F32)[:, NB:2 * NB]        # [128, NB] f32

                # --- Gl [128e, NB*F] f32 ---
                GRP = 512 // F
                ngrp = (NB + GRP - 1) // GRP
                gl = sb.tile([128, NB * F], F32, tag="gl")
                if layer == 1:
                    xt = sb.tile([128, ET], F32, tag="xt")
                    nc.sync.dma_start(out=xt[:], in_=xeT[:, bass.ds(i * ET, ET)])
                    for g in range(ngrp):
                        nblk = min(GRP, NB - g * GRP)
                        gp = p2.tile([128, 512], F32, tag="pbig", space="PSUM")
                        for jj in range(nblk):
                            j = g * GRP + jj
                            nc.tensor.matmul(
                                gp[:, jj * F:(jj + 1) * F],
                                lhsT=xt[:, j * 128:(j + 1) * 128],
                                rhs=wl1_s[:], start=True, stop=True)
                        nc.vector.tensor_copy(gl[:, g * GRP * F:(g * GRP + nblk) * F],
                                              gp[:, :nblk * F])
                elif USE_DMA_GATHER:
                    tbl = xl2 if layer == 2 else xl3
                    gi = sb.tile([128, (CA + CB) * 8], I16, tag="gi")
                    nc.sync.dma_start(out=gi[:], in_=gidx[bass.ds(i * 128, 128), :])
                    gl3 = gl[:].rearrange("p (b f) -> p b f", b=NB)
                    nc.gpsimd.dma_gather(gl3[:, :CA, :], tbl[:, :],
                                         gi[:, :CA * 8], CA * 128, CA * 128, F)
                    nc.gpsimd.dma_gather(gl3[:, CA:, :], tbl[BETA:, :],
                                         gi[:, CA * 8:], CB * 128, CB * 128, F)
                else:
                    tbl = xl2 if layer == 2 else xl3
                    for j in range(NB):
                        nc.gpsimd.indirect_dma_start(
                            out=gl[:, j * F:(j + 1) * F], out_offset=None,
                            in_=tbl[:],
                            in_offset=bass.IndirectOffsetOnAxis(ap=ss[:, j:j + 1], axis=0),
                            bounds_check=NTAB - 1, oob_is_err=False)

                # --- segment matrix S [128e, NB, SW] f16 ---
                S = sb.tile([128, NB, SW], F16, tag="S")
                nc.vector.tensor_tensor(
                    out=S[:],
                    in0=segf.rearrange("p (b one) -> p b one", one=1).to_broadcast([128, NB, SW]),
                    in1=iota_s[:, :SW].rearrange("p (one s) -> p one s", one=1)
                        .to_broadcast([128, NB, SW]),
                    op=ALU.is_equal)

                # --- S_T via PE transpose: st [SW, NB*128] f16 ---
                st = sb.tile([SW, NB * 128], F16, tag="st")
                nsgrp = (NB + 3) // 4
                for g in range(nsgrp):
                    nblk = min(4, NB - g * 4)
                    sp = p2.tile([SW, 512], F16, tag="psp", space="PSUM")
                    for jj in range(nblk):
                        j = g * 4 + jj
                        nc.tensor.transpose(sp[:, jj * 128:(jj + 1) * 128],
                                            S[:, j, :], ident_s[:])
                    nc.vector.tensor_copy(st[:, g * 512:g * 512 + nblk * 128],
                                          sp[:, :nblk * 128])

                # --- xr_u rows (hi/lo) for this tile's dsts ---
                hi_t, lo_t = xr_hi[layer - 1], xr_lo[layer - 1]
                xu = sb.tile([SW, 2 * HID], F16, tag="xu")
                nc.sync.dma_start(out=xu[:, :F], in_=hi_t[bass.ds(i * SEG, SW), :F])
                nc.sync.dma_start(out=xu[:, HID:HID + F], in_=lo_t[bass.ds(i * SEG, SW), :F])

                # --- m = Gl + S_T.T @ (xr_hi + xr_lo); leaky-relu (DVE) ---
                lr = sb.tile([128, NB * F], F32, tag="lr")
                for g in range(ngrp):
                    nblk = min(GRP, NB - g * GRP)
                    mp = p2.tile([128, 512], F32, tag="pbig", space="PSUM")
                    for jj in range(nblk):
                        j = g * GRP + jj
                        nc.tensor.matmul(mp[:, jj * F:(jj + 1) * F],
                                         lhsT=st[:, j * 128:(j + 1) * 128],
                                         rhs=xu[:, :F], start=True, stop=False)
                        nc.tensor.matmul(mp[:, jj * F:(jj + 1) * F],
                                         lhsT=st[:, j * 128:(j + 1) * 128],
                                         rhs=xu[:, HID:HID + F], start=False, stop=True)
                    gsl = slice(g * GRP * F, (g * GRP + nblk) * F)
                    msb = sb.tile([128, 512], F32, tag="msb")
                    nc.vector.tensor_add(msb[:, :nblk * F], gl[:, gsl],
                                         mp[:, :nblk * F])
                    nc.vector.scalar_tensor_tensor(
                        out=lr[:, gsl], in0=msb[:, :nblk * F], scalar=NEG_SLOPE,
                        in1=msb[:, :nblk * F], op0=ALU.mult, op1=ALU.max)

                # --- e = per-head dot with att (DVE), exp (ACT) ---
                C = F // HEADS
                tm = sb.tile([128, NB * F], F32, tag="tm")
                ew.tensor_tensor(
                    out=tm[:].rearrange("p (b h c) -> p b h c", b=NB, h=HEADS),
                    in0=lr[:].rearrange("p (b h c) -> p b h c", b=NB, h=HEADS),
                    in1=attr_s[:].rearrange("p (one h c) -> p one h c", one=1, h=HEADS)
                        .to_broadcast([128, NB, HEADS, C]),
                    op=ALU.mult)
                ev = sb.tile([128, NB * HEADS], F32, tag="ev")
                nc.vector.reduce_sum(
                    ev[:].rearrange("p (b h one) -> p b h one", b=NB, one=1),
                    tm[:].rearrange("p (b h c) -> p b h c", b=NB, h=HEADS),
                    axis=AX.X)
                exs = sb.tile([128, NB * HEADS], F16, tag="exs")
                nc.scalar.activation(exs[:], ev[:], ACTF.Exp)

                # --- P = Gl * ex (head-broadcast), f16 ---
                pp = sb.tile([128, NB * F], F16, tag="pp")
                ew.tensor_tensor(
                    out=pp[:].rearrange("p (b h c) -> p b h c", b=NB, h=HEADS),
                    in0=gl[:].rearrange("p (b h c) -> p b h c", b=NB, h=HEADS),
                    in1=exs[:].rearrange("p (b h one) -> p b h one", b=NB, one=1)
                        .to_broadcast([128, NB, HEADS, C]),
                    op=ALU.mult)

                # --- segment sums ---
                otp = p2.tile([F, SW], F32, tag="otp", space="PSUM")
                dnp = p1.tile([HEADS, SW], F32, tag="dnp", space="PSUM")
                for j in range(NB):
                    nc.tensor.matmul(dnp[:], lhsT=exs[:, j * HEADS:(j + 1) * HEADS],
                                     rhs=S[:, j, :],
                                     start=(j == 0), stop=(j == NB - 1))
                for j in range(NB):
                    nc.tensor.matmul(otp[:], lhsT=pp[:, j * F:(j + 1) * F],
                                     rhs=S[:, j, :],
                                     start=(j == 0), stop=(j == NB - 1))
                rd = sb.tile([HEADS, SW], F32, tag="rd")
                nc.vector.reciprocal(rd[:], dnp[:])
                dxp = p1.tile([F, SW], F32, tag="dxp", space="PSUM")
                e4c = e4_s if layer < 3 else e43_s
                nc.tensor.matmul(dxp[:], lhsT=e4c[:], rhs=rd[:], start=True, stop=True)
                dxs = sb.tile([F, SW], F32, tag="dxs")
                nc.vector.tensor_copy(dxs[:], dxp[:])
                z = sb.tile([F, SW], F32, tag="z")
                nc.vector.tensor_tensor(out=z[:], in0=otp[:], in1=dxs[:], op=ALU.mult)

                if layer < 3:
                    bc = b1c_s if layer == 1 else b2c_s
                    t1 = sb.tile([F, SEG], F32, tag="t1")
                    nc.vector.scalar_tensor_tensor(
                        out=t1[:], in0=z[:, :SEG], scalar=bc[:, :1],
                        in1=zeros_s[:F, :SEG], op0=ALU.add, op1=ALU.min)
                    t2 = sb.tile([F, SEG], F32, tag="t2")
                    nc.scalar.activation(t2[:], t1[:], ACTF.Exp)
                    t3 = sb.tile([F, SEG], F32, tag="t3")
                    nc.vector.scalar_tensor_tensor(
                        out=t3[:], in0=z[:, :SEG], scalar=bc[:, :1],
                        in1=zeros_s[:F, :SEG], op0=ALU.add, op1=ALU.max)
                    nc.vector.scalar_tensor_tensor(
                        out=hT[:, bass.ds(i * SEG, SEG)], in0=t2[:], scalar=-1.0,
                        in1=t3[:], op0=ALU.add, op1=ALU.add)
                else:
                    nc.sync.dma_start(out=opreT[:, bass.ds(i * SEG, SEG)],
                                      in_=z[:, :SEG])

            # ================= table phases =================
            def xr_phase(wr_s, li, F):
                hi_t, lo_t = xr_hi[li], xr_lo[li]
                for c in range(NP // 128):
                    p = p2.tile([128, F], F32, tag="pbig", space="PSUM")
                    nc.tensor.matmul(p[:], lhsT=hT[:, c * 128:(c + 1) * 128],
                                     rhs=wr_s[:], start=True, stop=True)
                    shi = sb.tile([128, F], F16, tag="shi")
                    nc.vector.tensor_copy(shi[:], p[:])
                    slo = sb.tile([128, F], F16, tag="slo")
                    nc.vector.tensor_tensor(out=slo[:], in0=p[:], in1=shi[:],
                                            op=ALU.subtract)
                    nc.sync.dma_start(out=hi_t[c * 128:(c + 1) * 128, :F], in_=shi[:])
                    nc.sync.dma_start(out=lo_t[c * 128:(c + 1) * 128, :F], in_=slo[:])

            def xl_phase(cc_out, wl_s, xlt, F):
                nchunks = NP // 128
                CG = 512 // F          # chunks whose outputs share one PSUM bank
                for r in range(NCORE):
                    for g0 in range(0, nchunks, CG):
                        ncg = min(CG, nchunks - g0)
                        htc = sb.tile([128, CG * 128], F32, tag="htc")
                        nc.sync.dma_start(
                            out=htc[:, :ncg * 128],
                            in_=cc_out[r * 128:(r + 1) * 128,
                                       g0 * 128:(g0 + ncg) * 128])
                        p = p2.tile([128, 512], F32, tag="pbig", space="PSUM")
                        for k in range(ncg):
                            nc.tensor.matmul(p[:, k * F:(k + 1) * F],
                                             lhsT=htc[:, k * 128:(k + 1) * 128],
                                             rhs=wl_s[:], start=True, stop=True)
                        s = sb.tile([128, 512], F32, tag="xls")
                        nc.vector.tensor_copy(s[:, :ncg * F], p[:, :ncg * F])
                        dview = xlt[r * NP + g0 * 128:r * NP + (g0 + ncg) * 128, :] \
                            .rearrange("(c p) f -> p c f", c=ncg)
                        nc.sync.dma_start(
                            out=dview,
                            in_=s[:, :ncg * F].rearrange("p (c f) -> p c f", c=ncg))

            def edge_loop(layer):
                with tc.For_i(0, NT, FOR_UNROLL, staggered_reset=STAGGER) as i0:
                    for u in range(FOR_UNROLL):
                        edge_tile(i0 + u, layer)

            # ================= layers =================
            nc.sync.dma_start(out=hT[:], in_=xT_loc[:])
            xr_phase(wr1_s, 0, HID)

            edge_loop(1)

            nc.sync.dma_start(out=cc_in1[:].rearrange("(p n) -> p n", p=128), in_=hT[:])
            nc.gpsimd.collective_compute(
                "AllGather", ALU.bypass,
                ins=[cc_in1.opt()], outs=[cc_out1.opt()],
                replica_groups=[list(range(NCORE))])
            xr_phase(wr2_s, 1, HID)
            xl_phase(cc_out1, wl2_s, xl2, HID)

            edge_loop(2)

            nc.sync.dma_start(out=cc_in2[:].rearrange("(p n) -> p n", p=128), in_=hT[:])
            nc.gpsimd.collective_compute(
                "AllGather", ALU.bypass,
                ins=[cc_in2.opt()], outs=[cc_out2.opt()],
                replica_groups=[list(range(NCORE))])
            xr_phase(wr3_s, 2, FINAL)
            xl_phase(cc_out2, wl3_s, xl3, FINAL)

            edge_loop(3)

            # ================= final: bias + log_softmax =================
            for c in range(NP // 128):
                ot = sb.tile([FINAL, 128], F32, tag="ot")
                nc.sync.dma_start(out=ot[:], in_=opreT[:, c * 128:(c + 1) * 128])
                tp = p2.tile([128, FINAL], F32, tag="pbig", space="PSUM")
                nc.tensor.transpose(tp[:], ot[:], ident32_s[:FINAL, :FINAL])
                t = sb.tile([128, FINAL], F32, tag="t")
                nc.vector.tensor_add(t[:], tp[:], b3r_s[:])
                nmx = sb.tile([128, 1], F32, tag="nmx")
                nc.vector.reduce_max(nmx[:], t[:], axis=AX.X, negate=True)
                em = sb.tile([128, FINAL], F32, tag="em")
                sm = sb.tile([128, 1], F32, tag="sm")
                nc.scalar.activation(em[:], t[:], ACTF.Exp, bias=nmx[:, :1],
                                     accum_out=sm[:])
                ln = sb.tile([128, 1], F32, tag="ln")
                nc.scalar.activation(ln[:], sm[:], ACTF.Ln)
                o = sb.tile([128, FINAL], F32, tag="o")
                nc.vector.scalar_tensor_tensor(
                    out=o[:], in0=t[:], scalar=nmx[:, :1],
                    in1=ln[:].to_broadcast([128, FINAL]),
                    op0=ALU.add, op1=ALU.subtract)
                nc.sync.dma_start(out=outp[c * 128:(c + 1) * 128, :], in_=o[:])

    nc.compile()
    return nc


_CACHED = {}
TRACE = False
LAST_EXEC_NS = None


def kernel(x, edge_index, Wl1, Wr1, att1, b1, Wl2, Wr2, att2, b2,
           Wl3, Wr3, att3, b3):
    per_core = _preprocess(x, edge_index)
    consts = _consts(Wl1, Wr1, att1, b1, Wl2, Wr2, att2, b2, Wl3, Wr3, att3, b3)

    if "nc" not in _CACHED:
        _CACHED["nc"] = _build()
    nc = _CACHED["nc"]

    in_maps = []
    for c in range(NCORE):
        m = dict(consts)
        m.update(per_core[c])
        in_maps.append(m)

    global LAST_EXEC_NS
    res = run_bass_kernel_spmd(nc, in_maps, core_ids=list(range(NCORE)),
                               trace=TRACE)
    LAST_EXEC_NS = res.exec_time_ns
    out = np.zeros((N, FINAL), dtype=np.float32)
    for c in range(NCORE):
        out[c * NLOC:(c + 1) * NLOC] = res.results[c]["outp"][:NLOC]
    return out
